# revision 19
# baseline (speedup 1.0000x reference)
"""APPNP GNN kernel for 8 TRN2 NeuronCores — gather + on-chip reduce version.

Reference computation (N=100000 nodes, E=1600000 edges, K=5, alpha=0.5):
    h0 = x @ W1 + b1
    deg[d] = |in-edges(d)| + 1 (self loop); dinv = rsqrt(deg)
    5x: h = (1-a) * dinv * S(dinv * h) + a * h0     (S = adjacency sum + self)
    out = relu(h) @ W2 + b2

Per-core strategy (nodes row-sharded 12500/core padded to 12544):
  track g_t = dinv*h_t.  Per iteration:
    AllGather g (3.2MB/core) -> g_full [100352, 64] f32 in local HBM.
    g_full is split in 4 chunks of 25088 rows (int16 gather index range).
    For each chunk c: the core's dsts are degree-sorted (by their in-edge
    count from chunk c) into 98 tiles of 128; a dma_gather on SWDGE queue c
    pulls each tile's messages into SBUF as [128 dst, D, 64] (pads read a
    known zero row), an in-place DVE pair-tree reduces over D, and the sync
    engine (HWDGE, no Q7 descriptors) writes the per-chunk partial rows
    [12544, 64] to HBM.
    Level 2: every node has exactly one partial per chunk; two gathers per
    region (chunks {0,1} / {2,3}, 2 partials per node each) land them
    tile-aligned and DVE combines  msum = A0+A1+B0+B1 + g (self loop), then
    g' = (1-a)*dinv^2*msum + a*g0  (fold: dinv2h*msum + g0a).
  All reduction is deterministic f32 on DVE; no dma_scatter_add anywhere.
  Q7 descriptor-generation (~7ns/desc) is the bottleneck; 4 SWDGE queues
  (8 Q7 cores) run desc-gen in parallel.
  Epilogue: u = relu(dinvh*msum + a*h0); out = u @ W2 + b2 (PE transpose).
"""

import math
import numpy as np

# ----------------------------------------------------------------- config

class Cfg:
    def __init__(self, N=100000, E=1600000, F=500, H=64, O=40, K=5, alpha=0.5,
                 cores=8, batch_slots=4096):
        self.N, self.E, self.F, self.H, self.O, self.K = N, E, F, H, O, K
        self.alpha = alpha
        self.cores = cores
        assert N % cores == 0
        self.NP = N // cores                      # real nodes per core
        self.PL = ((self.NP + 127) // 128) * 128  # padded nodes per core
        self.T = self.PL // 128                   # tiles per core
        self.FP = ((F + 127) // 128) * 128        # padded feature dim
        self.KT = self.FP // 128                  # k tiles for lin1
        self.NCH = 4                              # g_full chunks (int16 range)
        self.CH = cores * self.PL // self.NCH     # chunk rows
        assert self.CH <= 32600 and self.CH % self.PL == 0
        self.GR = cores * self.PL                 # g_full rows
        self.DMA_SCRATCH = 32768                  # SWDGE ring carveout bytes
        self.NQ = 4                               # SWDGE queues
        self.NS = 5                               # main gather slot ring
        self.SLOTC = 40                           # columns per gather slot
        self.BS = batch_slots                     # max slots per gather call
        assert self.BS <= 128 * self.SLOTC
        self.TG = 8                               # level-2 tile group
        self.LS = 2                               # level-2 slots per region

FULL = Cfg()

# ----------------------------------------------------------- host preprocess

def _wrap16(arr):
    w = arr.reshape(-1, 16).T.astype(np.int16)       # [16, S/16]
    return np.ascontiguousarray(np.tile(w, (8, 1)))  # [128, S/16]


def build_plan(cfg, edge_index):
    """Shared (core-independent) call structure + per-core index tables.

    Returns (plan, gidx_all, lidx_all) where plan.calls is the main gather
    call list [(chunk, col_off, ncols, D, nt)], plan.l2 the level-2 call list
    [(region, t0, nt, off)], and gidx/lidx the per-core slot index tables.
    """
    N, PL, NP, CH = cfg.N, cfg.PL, cfg.NP, cfg.CH
    src = np.asarray(edge_index[0], dtype=np.int64)
    dst = np.asarray(edge_index[1], dtype=np.int64)

    core_of_dst = dst // NP
    dst_loc = dst % NP
    src_pad = (src // NP) * PL + (src % NP)
    chunk = src_pad // CH
    src_loc = (src_pad % CH).astype(np.int64)

    # per (core, chunk, dst) degrees
    key = (core_of_dst * cfg.NCH + chunk) * PL + dst_loc
    counts = np.bincount(key, minlength=cfg.cores * cfg.NCH * PL)
    d = counts.reshape(cfg.cores, cfg.NCH, PL)

    # degree-descending order per (core, chunk); pos = rank of node
    ordr = np.argsort(-d, axis=2, kind="stable")           # [co, c, PL]
    pos = np.empty_like(ordr)
    ar = np.arange(PL)
    for co in range(cfg.cores):
        for c in range(cfg.NCH):
            pos[co, c, ordr[co, c]] = ar

    # shared per-tile degree: max over cores of tile-max (desc sort -> first)
    dsort = np.take_along_axis(d, ordr, axis=2)            # [co, c, PL] desc
    Dtile = dsort[:, :, ::128].max(axis=0)                 # [c, T]
    Dtile = np.maximum(Dtile, 1)
    maxD = int(Dtile.max())
    assert maxD <= cfg.SLOTC, f"tile degree {maxD} exceeds slot cols"

    # main gather calls: runs of equal-D tiles, <= BS slots (SLOTC cols)
    calls = []           # (chunk, col_off, ncols, D, nt, t0)
    colbase = np.zeros((cfg.NCH, cfg.T), dtype=np.int64)
    col = 0
    for c in range(cfg.NCH):
        t = 0
        while t < cfg.T:
            D = int(Dtile[c, t])
            nt = 1
            while (t + nt < cfg.T and int(Dtile[c, t + nt]) == D
                   and (nt + 1) * D <= cfg.SLOTC
                   and (nt + 1) * D * 128 <= cfg.BS):
                nt += 1
            colbase[c, t:t + nt] = col + np.arange(nt) * D
            calls.append((c, col, nt * D, D, nt, t))
            col += nt * D
            t += nt
    Ctot = col
    S1 = 128 * Ctot

    # per-core main index tables
    # occurrence rank within (core, chunk, dst)
    o1 = np.argsort(key, kind="stable")
    ks = key[o1]
    first = np.r_[True, ks[1:] != ks[:-1]]
    starts = np.where(first, np.arange(len(ks)), 0)
    occ = np.arange(len(ks)) - np.maximum.accumulate(starts)
    # back to edge order
    occ_e = np.empty_like(occ)
    occ_e[o1] = occ

    ZR = PL - 1                                   # zero row (pad) per chunk
    gidx_all = []
    rank_e = pos[core_of_dst, chunk, dst_loc]     # rank of dst in (co,chunk)
    t_e = rank_e // 128
    p_e = rank_e % 128
    slot_e = (colbase[chunk, t_e] + occ_e) * 128 + p_e
    for co in range(cfg.cores):
        flat = np.full(S1, ZR, dtype=np.int16)
        m = core_of_dst == co
        flat[slot_e[m]] = src_loc[m].astype(np.int16)
        gidx_all.append(_wrap16(flat))

    # level-2: region r, natural tile t, k in {0,1} -> chunk 2r+k
    L2N = 2 * cfg.T * 2 * 128                     # slots per region
    l2_calls = []
    for r in range(2):
        t0 = 0
        while t0 < cfg.T:
            nt = min(cfg.TG, cfg.T - t0)
            l2_calls.append((r, t0, nt, r * L2N // 2 + t0 * 2 * 128))
            t0 += nt
    lidx_all = []
    nodes = np.arange(PL)
    for co in range(cfg.cores):
        flat = np.empty(L2N, dtype=np.int16)
        for r in range(2):
            for k in range(2):
                v = (k * PL + pos[co, 2 * r + k, nodes]).astype(np.int16)
                # slot ((t*2+k)*128 + p) ; node = t*128+p
                sl = ((nodes // 128) * 2 + k) * 128 + nodes % 128
                flat[r * (L2N // 2) + sl] = v
        lidx_all.append(_wrap16(flat))

    class Plan:
        pass
    plan = Plan()
    plan.calls = calls
    plan.l2 = l2_calls
    plan.S1 = S1
    plan.L2N = L2N
    plan.maxD = maxD
    plan.pad_frac = (S1 - int(np.sum(d)) / cfg.cores) / max(S1, 1)
    return plan, gidx_all, lidx_all


def host_prep(cfg, x, edge_index, W1, b1, W2, b2):
    N, H, F = cfg.N, cfg.H, cfg.F
    dst = np.asarray(edge_index[1], dtype=np.int64)
    deg = np.bincount(dst, minlength=N).astype(np.float64) + 1.0
    dinv = (1.0 / np.sqrt(deg)).astype(np.float32)

    plan, gidx_all, lidx_all = build_plan(cfg, edge_index)

    def tileize(v):  # [PL] -> [128, T]
        return np.ascontiguousarray(v.reshape(cfg.T, 128).T)

    xT = np.zeros((cfg.FP, cfg.PL), dtype=np.float32)
    W1p = np.zeros((cfg.FP, H), dtype=np.float32)
    W1p[:F] = W1.astype(np.float32)
    b1r = np.ascontiguousarray(np.broadcast_to(b1.astype(np.float32), (128, H)))
    b2r = np.ascontiguousarray(np.broadcast_to(b2.astype(np.float32), (128, cfg.O)))
    ident = np.eye(128, dtype=np.float32)

    in_maps = []
    for c in range(cfg.cores):
        xs = x[c * cfg.NP:(c + 1) * cfg.NP].astype(np.float32)
        xTc = xT.copy()
        xTc[:F, :cfg.NP] = xs.T
        dv = np.zeros(cfg.PL, dtype=np.float32)
        dv[:cfg.NP] = dinv[c * cfg.NP:(c + 1) * cfg.NP]
        rdv = np.zeros(cfg.PL, dtype=np.float32)
        rdv[:cfg.NP] = 1.0 / dv[:cfg.NP]
        in_maps.append({
            "xT": xTc,
            "w1": W1p,
            "b1r": b1r,
            "w2": W2.astype(np.float32),
            "b2r": b2r,
            "ident": ident,
            "dinv": tileize(dv),
            "dinvh": tileize((1.0 - cfg.alpha) * dv),
            "dinv2h": tileize((1.0 - cfg.alpha) * dv * dv),
            "rdinv": tileize(rdv),
            "gidx": gidx_all[c],
            "lidx": lidx_all[c],
        })
    return in_maps, plan


# ------------------------------------------------------------- graph builder

def build_graph(cfg, plan, compile_for_hw=True):
    import concourse.bass as bass
    import concourse.bacc as bacc
    import concourse.mybir as mybir
    from concourse.library_config import mlp

    f32 = mybir.dt.float32
    i16 = mybir.dt.int16
    H, O, T, PL = cfg.H, cfg.O, cfg.T, cfg.PL
    NS, NQ, LS, TG = cfg.NS, cfg.NQ, cfg.LS, cfg.TG
    S1, L2N = plan.S1, plan.L2N
    S16, L16 = S1 // 16, L2N // 16

    # round-robin main calls across chunks (queue = chunk)
    per_chunk = [[b for b in plan.calls if b[0] == c] for c in range(cfg.NCH)]
    order = []
    i = 0
    while any(i < len(pc) for pc in per_chunk):
        for c in range(cfg.NCH):
            if i < len(per_chunk[c]):
                order.append(per_chunk[c][i])
        i += 1
    CALLS = order
    NC = len(CALLS)
    # last issue position of a chunk-{0,1} call (region-A barrier point)
    last01 = max(k for k, b in enumerate(CALLS) if b[0] < 2)

    # level-2 calls interleaved A0,B0,A1,B1,... ; regions have equal length
    l2A = [b for b in plan.l2 if b[0] == 0]
    l2B = [b for b in plan.l2 if b[0] == 1]
    assert len(l2A) == len(l2B)
    L2C = [x for pair in zip(l2A, l2B) for x in pair]
    NL = len(L2C)
    NLR = len(l2A)         # calls per region

    # -------- static schedule: absolute per-slot semaphore counts ----------
    # main slots: gd (gather landed), dv (tree done), pw (partial written)
    gd_tot = [0] * NS
    dv_tot = [0] * NS
    pw_tot = [0] * NS
    lg_tot = [0] * (2 * LS)
    MAIN = []              # MAIN[t][k] = (b, s, gd_prior, dv_prior, pw_prior)
    BARA = []              # BARA[t] = (slot, pw_target) region-A barrier
    BARB = []              # BARB[t] = (slot, pw_target) region-B barrier
    L2S = []               # L2S[t][j] = (b, sl, lg_prior)
    for t in range(cfg.K):
        it = []
        for k, b in enumerate(CALLS):
            s = (t * NC + k) % NS
            it.append((b, s, gd_tot[s], dv_tot[s], pw_tot[s]))
            gd_tot[s] += 1
            if b[3] >= 2:
                dv_tot[s] += 1
            pw_tot[s] += 1
            if k == last01:
                BARA.append((s, pw_tot[s]))
        BARB.append(((t * NC + NC - 1) % NS, pw_tot[(t * NC + NC - 1) % NS]))
        MAIN.append(it)
        l2 = []
        for j, b in enumerate(L2C):
            r = b[0]
            jj = j // 2                       # index within region
            sl = r * LS + jj % LS
            l2.append((b, sl, lg_tot[sl]))
            lg_tot[sl] += 1
        L2S.append(l2)

    nc = bacc.Bacc("TRN2", target_bir_lowering=False, debug=False,
                   num_devices=cfg.cores, num_swdge_queues=NQ,
                   dynamic_dma_scratch_size=cfg.DMA_SCRATCH)

    xT_h = nc.declare_dram_parameter("xT", [cfg.FP, PL], f32, isOutput=False)
    w1_h = nc.declare_dram_parameter("w1", [cfg.FP, H], f32, isOutput=False)
    b1r_h = nc.declare_dram_parameter("b1r", [128, H], f32, isOutput=False)
    w2_h = nc.declare_dram_parameter("w2", [H, O], f32, isOutput=False)
    b2r_h = nc.declare_dram_parameter("b2r", [128, O], f32, isOutput=False)
    id_h = nc.declare_dram_parameter("ident", [128, 128], f32, isOutput=False)
    dinv_h = nc.declare_dram_parameter("dinv", [128, T], f32, isOutput=False)
    dinvh_h = nc.declare_dram_parameter("dinvh", [128, T], f32, isOutput=False)
    dinv2h_h = nc.declare_dram_parameter("dinv2h", [128, T], f32, isOutput=False)
    rdinv_h = nc.declare_dram_parameter("rdinv", [128, T], f32, isOutput=False)
    gidx_h = nc.declare_dram_parameter("gidx", [128, S16], i16, isOutput=False)
    lidx_h = nc.declare_dram_parameter("lidx", [128, L16], i16, isOutput=False)
    out_h = nc.declare_dram_parameter("out", [T, 128, O], f32, isOutput=True)

    cc_in = nc.dram_tensor("cc_in", [PL, H], f32)
    g_full = nc.dram_tensor("g_full", [cfg.GR, H], f32, addr_space="Shared")
    partials = nc.dram_tensor("partials", [cfg.NCH * PL, H], f32)

    # lin1 m-groups
    MGW = []
    rem = PL
    while rem > 0:
        w = min(512, rem)
        MGW.append(w)
        rem -= w

    rg = [list(range(cfg.cores))]

    ctxs = []

    def sb(name, shape, dtype):
        cm = nc.sbuf_tensor(name, shape, dtype)
        h = cm.__enter__()
        ctxs.append(cm)
        return h

    def ps(name, shape, dtype):
        cm = nc.psum_tensor(name, shape, dtype)
        h = cm.__enter__()
        ctxs.append(cm)
        return h

    def sem(name):
        cm = nc.semaphore(name)
        h = cm.__enter__()
        ctxs.append(cm)
        return h

    SLOT_F32 = max(cfg.SLOTC * H, 512 * cfg.KT)   # per-partition f32 per slot
    stage = sb("stage", [128, NS * SLOT_F32], f32)
    l2st = sb("l2st", [128, 2 * LS * TG * 2 * H], f32)   # region-major
    idxg_sb = sb("idxg", [128, S16], i16)
    idxl_sb = sb("idxl", [128, L16], i16)
    g0a_sb = sb("g0a", [128, T, H], f32)
    g_sb = sb("g", [128, T, H], f32)
    msum_sb = sb("msum_sb", [128, T, H], f32)
    w1_sb = sb("w1_sb", [128, cfg.KT, H], f32)
    w2_sb = sb("w2_sb", [H, O], f32)
    b1r_sb = sb("b1r_sb", [128, H], f32)
    b2r_sb = sb("b2r_sb", [128, O], f32)
    id_sb = sb("id_sb", [128, 128], f32)
    dinv_sb = sb("dinv_sb", [128, T], f32)
    dinvh_sb = sb("dinvh_sb", [128, T], f32)
    dinv2h_sb = sb("dinv2h_sb", [128, T], f32)
    rdinv_sb = sb("rdinv_sb", [128, T], f32)
    tmp_sb = sb("tmp_sb", [128, 2, H], f32)
    u_sb = sb("u_sb", [128, 2, H], f32)
    ur_sb = sb("ur_sb", [128, 2, H], f32)
    ah_sb = sb("ah_sb", [128, 2, H], f32)
    lhsT_sb = sb("lhsT_sb", [H, 2, 128], f32)
    OG = 14                                   # out-group tiles
    out_sb = sb("out_sb", [128, 2, OG, O], f32)

    ps_mm = [ps("ps_mm0", [128, H], f32), ps("ps_mm1", [128, H], f32)]
    ps_tr = [ps("ps_tr0", [H, 128], f32), ps("ps_tr1", [H, 128], f32)]
    ps_o = [ps("ps_o0", [128, O], f32), ps("ps_o1", [128, O], f32)]

    s_in = sem("s_in")
    s_x = [sem("s_x0"), sem("s_x1")]
    s_mm = sem("s_mm")
    s_ppf = sem("s_ppf")
    s_ep = sem("s_ep")
    s_gw = sem("s_gw")
    s_cc = sem("s_cc")
    s_gd = [sem(f"s_gd{i}") for i in range(NS)]   # gather landed (16/call)
    s_dv = [sem(f"s_dv{i}") for i in range(NS)]   # DVE tree done (1/call)
    s_pw = [sem(f"s_pw{i}") for i in range(NS)]   # partial written (16/call)
    s_lg = [sem(f"s_lg{i}") for i in range(2 * LS)]  # L2 landed
    s_lc = [sem(f"s_lc{i}") for i in range(2 * LS)]  # L2 consumed
    s_gu = sem("s_gu")
    s_ah = sem("s_ah")
    s_u = sem("s_u")
    s_ur = sem("s_ur")
    s_tr = sem("s_tr")
    s_trc = sem("s_trc")
    s_mo = sem("s_mo")
    s_ob = sem("s_ob")
    s_ow = [sem("s_ow0"), sem("s_ow1")]

    def slot_view(s, nt, D):                  # [128, nt, D, H]
        base = s * SLOT_F32
        return stage[:, base:base + nt * D * H].rearrange(
            "p (n d m) -> p n d m", d=D, m=H)

    def slot_flat(s, ncols):                  # [128, ncols, H] gather target
        base = s * SLOT_F32
        return stage[:, base:base + ncols * H].rearrange(
            "p (n m) -> p n m", m=H)

    def l2_view(r, ls, nt):                   # [128, nt, 2, H]
        base = (r * LS + ls) * TG * 2 * H
        return l2st[:, base:base + nt * 2 * H].rearrange(
            "p (n k m) -> p n k m", k=2, m=H)

    def stage_slot_x(s, kt, w):               # [128, kt, w] lin1 xT view
        return stage[:, s * SLOT_F32:s * SLOT_F32 + kt * 512].rearrange(
            "p (k m) -> p k m", k=kt)[:, :, :w]

    xT3 = xT_h.ap().rearrange("(k p) m -> p k m", p=128)
    w13 = w1_h.ap().rearrange("(k p) m -> p k m", p=128)
    cc3 = cc_in.ap().rearrange("(n p) m -> p n m", p=128)

    n_in_dmas = 12

    tiles_per_group = [(w + 127) // 128 for w in MGW]
    cum_tiles = np.cumsum([0] + tiles_per_group)

    # ============================== setup block ==============================
    with nc.Block() as blk:
        @blk.sync
        def _(sy):
            sy.dma_start(idxg_sb[:, :], gidx_h[:, :]).then_inc(s_in, 16)
            sy.dma_start(idxl_sb[:, :], lidx_h[:, :]).then_inc(s_in, 16)
            sy.dma_start(w1_sb[:, :, :], w13).then_inc(s_in, 16)
            sy.dma_start(w2_sb[:, :], w2_h[:, :]).then_inc(s_in, 16)
            sy.dma_start(b1r_sb[:, :], b1r_h[:, :]).then_inc(s_in, 16)
            sy.dma_start(b2r_sb[:, :], b2r_h[:, :]).then_inc(s_in, 16)
            sy.dma_start(id_sb[:, :], id_h[:, :]).then_inc(s_in, 16)
            sy.dma_start(dinv_sb[:, :], dinv_h[:, :]).then_inc(s_in, 16)
            sy.dma_start(dinvh_sb[:, :], dinvh_h[:, :]).then_inc(s_in, 16)
            sy.dma_start(dinv2h_sb[:, :], dinv2h_h[:, :]).then_inc(s_in, 16)
            sy.dma_start(rdinv_sb[:, :], rdinv_h[:, :]).then_inc(s_in, 16)
            sy.wait_ge(s_in, 16 * (n_in_dmas - 1))
            col = 0
            for mg, w in enumerate(MGW):
                if mg >= 2:
                    sy.wait_ge(s_mm, int(cum_tiles[mg - 1]))
                sy.dma_start(stage_slot_x(mg % 2, cfg.KT, w),
                             xT3[:, :, col:col + w]).then_inc(s_x[mg % 2], 16)
                col += w
            sy.wait_ge(s_ep, T)
            sy.dma_start(cc3, g_sb[:, :, :]).then_inc(s_gw, 16)
            sy.wait_ge(s_gw, 16)

        @blk.tensor
        def _(pe):
            pe.wait_ge(s_in, 16 * (n_in_dmas - 1))
            ti = 0
            for mg, w in enumerate(MGW):
                pe.wait_ge(s_x[mg % 2], 16 * (mg // 2 + 1))
                nt = tiles_per_group[mg]
                for m in range(nt):
                    mw = min(128, w - m * 128)
                    if ti >= 2:
                        pe.wait_ge(s_ppf, ti - 1)
                    for k in range(cfg.KT):
                        ins = pe.matmul(
                            ps_mm[ti % 2][:mw, :],
                            stage_slot_x(mg % 2, cfg.KT, w)[:, k, m * 128:m * 128 + mw],
                            w1_sb[:, k, :],
                            start=(k == 0), stop=(k == cfg.KT - 1))
                        if k == cfg.KT - 1:
                            ins.then_inc(s_mm)
                    ti += 1

        @blk.vector
        def _(ve):
            ve.wait_ge(s_in, 16 * (n_in_dmas - 1))
            for ti in range(T):
                ve.wait_ge(s_mm, ti + 1)
                if ti >= 2:
                    ve.wait_ge(s_ep, ti - 1)   # Act consumed tmp slot
                ve.tensor_add(tmp_sb[:, ti % 2, :], ps_mm[ti % 2][:, :],
                              b1r_sb[:, :]).then_inc(s_ppf)

        @blk.scalar
        def _(ac):
            ac.wait_ge(s_in, 16 * (n_in_dmas - 1))
            for ti in range(T):
                ac.wait_ge(s_ppf, ti + 1)
                ac.activation(g0a_sb[:, ti, :], tmp_sb[:, ti % 2, :],
                              mybir.ActivationFunctionType.Copy,
                              scale=dinvh_sb[:, ti:ti + 1])
                ac.activation(g_sb[:, ti, :], tmp_sb[:, ti % 2, :],
                              mybir.ActivationFunctionType.Copy,
                              scale=dinv_sb[:, ti:ti + 1]).then_inc(s_ep)

        @blk.gpsimd
        def _(gp):
            gp.load_library(mlp)
            gp.wait_ge(s_gw, 16)
            gp.collective_compute(
                "AllGather", mybir.AluOpType.bypass, replica_groups=rg,
                ins=[cc_in.ap().opt()], outs=[g_full.ap().opt()],
            ).then_inc(s_cc)

    # ============================ iteration blocks ===========================
    for t in range(cfg.K):
        last = (t == cfg.K - 1)
        with nc.Block() as blk:
            @blk.gpsimd
            def _(gp, t=t, last=last):
                gp.wait_ge(s_cc, t + 1)
                # main gathers, queue = chunk
                for (c, coff, ncols, D, nt, t0), s, gdp, dvp, pwp in MAIN[t]:
                    if pwp > 0:
                        gp.wait_ge(s_pw[s], 16 * pwp)
                    nb = ncols * 128
                    off = coff * 128
                    gp.dma_gather(
                        slot_flat(s, ncols),
                        g_full[c * cfg.CH:(c + 1) * cfg.CH, :],
                        idxg_sb[:, off // 16:(off + nb) // 16],
                        nb, nb, H, elem_step=H, queue_num=c,
                        single_packet=False,
                    ).then_inc(s_gd[s], 16)
                # level-2 gathers: A on queues 0/1, B on 2/3, interleaved
                bar_done = [False, False]
                for j, ((r, t0, nt, loff), sl, lgp) in enumerate(L2S[t]):
                    if not bar_done[r]:
                        bs, btgt = (BARA[t] if r == 0 else BARB[t])
                        gp.wait_ge(s_pw[bs], 16 * btgt)
                        bar_done[r] = True
                    if lgp > 0:
                        gp.wait_ge(s_lc[sl], lgp)
                    nb = nt * 2 * 128
                    jj = j // 2
                    gp.dma_gather(
                        l2_view(r, sl - r * LS, nt).rearrange(
                            "p n k m -> p (n k) m"),
                        partials[2 * r * PL:(2 * r + 2) * PL, :],
                        idxl_sb[:, loff // 16:(loff + nb) // 16],
                        nb, nb, H, elem_step=H, queue_num=2 * r + (jj % 2),
                        single_packet=False,
                    ).then_inc(s_lg[sl], 16)
                if not last:
                    gp.wait_ge(s_gw, 16 * (t + 2))
                    gp.collective_compute(
                        "AllGather", mybir.AluOpType.bypass, replica_groups=rg,
                        ins=[cc_in.ap().opt()], outs=[g_full.ap().opt()],
                    ).then_inc(s_cc)

            @blk.vector
            def _(ve, t=t, last=last):
                # main tree reduces (issue order)
                for (c, coff, ncols, D, nt, t0), s, gdp, dvp, pwp in MAIN[t]:
                    ve.wait_ge(s_gd[s], 16 * (gdp + 1))
                    if D >= 2:
                        v = slot_view(s, nt, D)
                        cur = D
                        ins = None
                        while cur > 1:
                            h = cur // 2
                            lo = cur - h
                            ins = ve.tensor_add(v[:, :, 0:h, :],
                                                v[:, :, 0:h, :],
                                                v[:, :, lo:lo + h, :])
                            cur = lo
                        ins.then_inc(s_dv[s])
                # level-2 combines + self loop + g update
                for j, ((r, t0, nt, loff), sl, lgp) in enumerate(L2S[t]):
                    ve.wait_ge(s_lg[sl], 16 * (lgp + 1))
                    v = l2_view(r, sl - r * LS, nt)
                    mr = msum_sb[:, t0:t0 + nt, :]
                    if r == 0:
                        ve.tensor_add(mr, v[:, :, 0, :],
                                      v[:, :, 1, :]).then_inc(s_lc[sl])
                    else:
                        ve.tensor_add(mr, mr, v[:, :, 0, :])
                        ve.tensor_add(mr, mr, v[:, :, 1, :]).then_inc(s_lc[sl])
                        ve.tensor_add(mr, mr, g_sb[:, t0:t0 + nt, :])
                        if not last:
                            for ti in range(t0, t0 + nt):
                                ins = ve.scalar_tensor_tensor(
                                    g_sb[:, ti, :], msum_sb[:, ti, :],
                                    dinv2h_sb[:, ti:ti + 1], g0a_sb[:, ti, :],
                                    mybir.AluOpType.mult, mybir.AluOpType.add)
                                if ti == T - 1:
                                    ins.then_inc(s_gu)
                        else:
                            if t0 + nt == T:
                                ve.tensor_copy(tmp_sb[:, 0, :],
                                               msum_sb[:, 0, :]).then_inc(s_gu)

            @blk.sync
            def _(sy, t=t, last=last):
                # partial write-outs (issue order; HWDGE FIFO per engine)
                for (c, coff, ncols, D, nt, t0), s, gdp, dvp, pwp in MAIN[t]:
                    if D >= 2:
                        sy.wait_ge(s_dv[s], dvp + 1)
                    else:
                        sy.wait_ge(s_gd[s], 16 * (gdp + 1))
                    v = slot_view(s, nt, D)
                    dst = partials[c * PL + t0 * 128:
                                   c * PL + (t0 + nt) * 128, :].rearrange(
                                       "(n p) m -> p n m", p=128)
                    sy.dma_start(dst, v[:, :, 0, :]).then_inc(s_pw[s], 16)
                sy.wait_ge(s_gu, t + 1)
                if not last:
                    sy.dma_start(cc3, g_sb[:, :, :]).then_inc(s_gw, 16)
                    sy.wait_ge(s_gw, 16 * (t + 2))

    # ============================== epilogue =================================
    with nc.Block() as blk:
        @blk.scalar
        def _(ac):
            for ti in range(T):
                if ti >= 2:
                    ac.wait_ge(s_u, ti - 1)     # DVE consumed ah slot
                ac.activation(ah_sb[:, ti % 2, :], g0a_sb[:, ti, :],
                              mybir.ActivationFunctionType.Copy,
                              scale=rdinv_sb[:, ti:ti + 1]).then_inc(s_ah)
                ac.wait_ge(s_u, ti + 1)
                if ti >= 2:
                    ac.wait_ge(s_tr, ti - 1)    # PE consumed ur slot
                ac.activation(ur_sb[:, ti % 2, :], u_sb[:, ti % 2, :],
                              mybir.ActivationFunctionType.Relu).then_inc(s_ur)

        @blk.vector
        def _(ve):
            ve.wait_ge(s_gu, cfg.K)
            for ti in range(T):
                og, oslot = ti // OG, (ti // OG) % 2
                if ti % OG == 0 and og >= 2:
                    ve.wait_ge(s_ow[og % 2], 16 * (og // 2))
                ve.wait_ge(s_ah, ti + 1)
                if ti >= 2:
                    ve.wait_ge(s_ur, ti - 1)    # Act consumed u slot
                ve.scalar_tensor_tensor(
                    u_sb[:, ti % 2, :], msum_sb[:, ti, :],
                    dinvh_sb[:, ti:ti + 1], ah_sb[:, ti % 2, :],
                    mybir.AluOpType.mult, mybir.AluOpType.add).then_inc(s_u)
                ve.wait_ge(s_tr, ti + 1)
                ve.tensor_copy(lhsT_sb[:, ti % 2, :],
                               ps_tr[ti % 2][:, :]).then_inc(s_trc)
                ve.wait_ge(s_mo, ti + 1)
                ve.tensor_add(out_sb[:, oslot, ti % OG, :], ps_o[ti % 2][:, :],
                              b2r_sb[:, :]).then_inc(s_ob)

        @blk.tensor
        def _(pe):
            for ti in range(T):
                pe.wait_ge(s_ur, ti + 1)
                if ti >= 2:
                    pe.wait_ge(s_trc, ti - 1)
                pe.transpose(ps_tr[ti % 2][:, :], ur_sb[:, ti % 2, :],
                             id_sb[:, :]).then_inc(s_tr)
                pe.wait_ge(s_trc, ti + 1)
                if ti >= 2:
                    pe.wait_ge(s_ob, ti - 1)
                pe.matmul(ps_o[ti % 2][:, :], lhsT_sb[:, ti % 2, :],
                          w2_sb[:, :], start=True, stop=True).then_inc(s_mo)

        @blk.sync
        def _(sy):
            ngroups = (T + OG - 1) // OG
            for og in range(ngroups):
                t0 = og * OG
                nt = min(OG, T - t0)
                sy.wait_ge(s_ob, t0 + nt)
                dst = out_h[t0:t0 + nt, :, :].rearrange("n p m -> p n m")
                sy.dma_start(dst, out_sb[:, og % 2, 0:nt, :]).then_inc(
                    s_ow[og % 2], 16)
            for par in range(2):
                n_par = (ngroups + 1 - par) // 2
                if n_par:
                    sy.wait_ge(s_ow[par], 16 * n_par)

    print(f"SBUF used: {(nc.sbuf_base + (nc.SBUF_PARTITION_SIZE_BYTES - nc.sbuf_top)) / 1024:.0f} KB/part "
          f"(base {nc.sbuf_base//1024}KB top-res {(nc.SBUF_PARTITION_SIZE_BYTES - nc.sbuf_top)//1024}KB of {nc.SBUF_PARTITION_SIZE_BYTES//1024}KB) "
          f"NC={NC} NL={NL} S1={S1} pad={plan.pad_frac:.3f} maxD={plan.maxD}")
    if compile_for_hw:
        nc.compile()
    return nc


# ----------------------------------------------------------------- kernel()

_CACHE = {}


def _run(cfg, inputs, trace=False):
    from concourse.bass_utils import run_bass_kernel_spmd

    in_maps, plan = host_prep(cfg, inputs["x"], inputs["edge_index"],
                              inputs["W1"], inputs["b1"],
                              inputs["W2"], inputs["b2"])
    key = (cfg.N, cfg.E, plan.S1, tuple(b[:4] for b in plan.calls))
    if key not in _CACHE:
        _CACHE[key] = build_graph(cfg, plan)
    nc = _CACHE[key]
    res = run_bass_kernel_spmd(nc, in_maps, list(range(cfg.cores)), trace=trace)
    outs = []
    for c in range(cfg.cores):
        o = np.asarray(res.results[c]["out"]).reshape(cfg.PL, cfg.O)
        outs.append(o[:cfg.NP])
    return np.concatenate(outs, axis=0), res


def kernel(**inputs):
    out, _ = _run(FULL, inputs)
    return out


# revision 20
# speedup vs baseline: 1.0960x; 1.0960x over previous
"""APPNP GNN kernel for 8 TRN2 NeuronCores — gather + on-chip reduce version.

Reference computation (N=100000 nodes, E=1600000 edges, K=5, alpha=0.5):
    h0 = x @ W1 + b1
    deg[d] = |in-edges(d)| + 1 (self loop); dinv = rsqrt(deg)
    5x: h = (1-a) * dinv * S(dinv * h) + a * h0     (S = adjacency sum + self)
    out = relu(h) @ W2 + b2

Per-core strategy (nodes row-sharded 12500/core padded to 12544):
  track g_t = dinv*h_t.  Per iteration:
    AllGather g (3.2MB/core) -> g_full [100352, 64] f32 in local HBM.
    g_full is split in 4 chunks of 25088 rows (int16 gather index range).
    For each chunk c: the core's dsts are degree-sorted (by their in-edge
    count from chunk c) into 98 tiles of 128; a dma_gather on SWDGE queue c
    pulls each tile's messages into SBUF as [128 dst, D, 64] (pads read a
    known zero row), an in-place DVE pair-tree reduces over D, and the sync
    engine (HWDGE, no Q7 descriptors) writes the per-chunk partial rows
    [12544, 64] to HBM.
    Level 2: every node has exactly one partial per chunk; two gathers per
    region (chunks {0,1} / {2,3}, 2 partials per node each) land them
    tile-aligned and DVE combines  msum = A0+A1+B0+B1 + g (self loop), then
    g' = (1-a)*dinv^2*msum + a*g0  (fold: dinv2h*msum + g0a).
  All reduction is deterministic f32 on DVE; no dma_scatter_add anywhere.
  Q7 descriptor-generation (~7ns/desc) is the bottleneck; 4 SWDGE queues
  (8 Q7 cores) run desc-gen in parallel.
  Epilogue: u = relu(dinvh*msum + a*h0); out = u @ W2 + b2 (PE transpose).
"""

import math
import numpy as np

# ----------------------------------------------------------------- config

class Cfg:
    def __init__(self, N=100000, E=1600000, F=500, H=64, O=40, K=5, alpha=0.5,
                 cores=8, batch_slots=4096):
        self.N, self.E, self.F, self.H, self.O, self.K = N, E, F, H, O, K
        self.alpha = alpha
        self.cores = cores
        assert N % cores == 0
        self.NP = N // cores                      # real nodes per core
        self.PL = ((self.NP + 127) // 128) * 128  # padded nodes per core
        self.T = self.PL // 128                   # tiles per core
        self.FP = ((F + 127) // 128) * 128        # padded feature dim
        self.KT = self.FP // 128                  # k tiles for lin1
        self.NCH = 4                              # g_full chunks (int16 range)
        self.CH = cores * self.PL // self.NCH     # chunk rows
        assert self.CH <= 32600 and self.CH % self.PL == 0
        self.GR = cores * self.PL                 # g_full rows
        self.DMA_SCRATCH = 32768                  # SWDGE ring carveout bytes
        self.NQ = 4                               # SWDGE queues
        self.NS = 8                               # main slots: queue q owns {2q,2q+1}
        self.SLOTC = 24                           # columns per gather slot / call
        self.BS = batch_slots                     # max slots per gather call
        self.TG = 8                               # level-2 tile group
        self.LS = 2                               # level-2 slots per region

FULL = Cfg()

# ----------------------------------------------------------- host preprocess

def _wrap16(arr):
    w = arr.reshape(-1, 16).T.astype(np.int16)       # [16, S/16]
    return np.ascontiguousarray(np.tile(w, (8, 1)))  # [128, S/16]


def build_plan(cfg, edge_index):
    """Shared (core-independent) call structure + per-core index tables.

    Returns (plan, gidx_all, lidx_all) where plan.calls is the main gather
    call list [(chunk, col_off, ncols, D, nt)], plan.l2 the level-2 call list
    [(region, t0, nt, off)], and gidx/lidx the per-core slot index tables.
    """
    N, PL, NP, CH = cfg.N, cfg.PL, cfg.NP, cfg.CH
    src = np.asarray(edge_index[0], dtype=np.int64)
    dst = np.asarray(edge_index[1], dtype=np.int64)

    core_of_dst = dst // NP
    dst_loc = dst % NP
    src_pad = (src // NP) * PL + (src % NP)
    chunk = src_pad // CH
    src_loc = (src_pad % CH).astype(np.int64)

    # per (core, chunk, dst) degrees
    key = (core_of_dst * cfg.NCH + chunk) * PL + dst_loc
    counts = np.bincount(key, minlength=cfg.cores * cfg.NCH * PL)
    d = counts.reshape(cfg.cores, cfg.NCH, PL)

    # degree-descending order per (core, chunk); pos = rank of node
    ordr = np.argsort(-d, axis=2, kind="stable")           # [co, c, PL]
    pos = np.empty_like(ordr)
    ar = np.arange(PL)
    for co in range(cfg.cores):
        for c in range(cfg.NCH):
            pos[co, c, ordr[co, c]] = ar

    # shared per-tile degree: max over cores of tile-max (desc sort -> first)
    dsort = np.take_along_axis(d, ordr, axis=2)            # [co, c, PL] desc
    Dtile = dsort[:, :, ::128].max(axis=0)                 # [c, T]
    Dtile = np.maximum(Dtile, 1)
    maxD = int(Dtile.max())
    assert maxD <= cfg.SLOTC, f"tile degree {maxD} exceeds slot cols"

    # main gather calls: runs of equal-D tiles, <= BS slots (SLOTC cols)
    calls = []           # (chunk, col_off, ncols, D, nt, t0)
    colbase = np.zeros((cfg.NCH, cfg.T), dtype=np.int64)
    col = 0
    for c in range(cfg.NCH):
        t = 0
        while t < cfg.T:
            D = int(Dtile[c, t])
            nt = 1
            while (t + nt < cfg.T and int(Dtile[c, t + nt]) == D
                   and (nt + 1) * D <= cfg.SLOTC
                   and (nt + 1) * D * 128 <= cfg.BS):
                nt += 1
            colbase[c, t:t + nt] = col + np.arange(nt) * D
            calls.append((c, col, nt * D, D, nt, t))
            col += nt * D
            t += nt
    Ctot = col
    S1 = 128 * Ctot

    # per-core main index tables
    # occurrence rank within (core, chunk, dst)
    o1 = np.argsort(key, kind="stable")
    ks = key[o1]
    first = np.r_[True, ks[1:] != ks[:-1]]
    starts = np.where(first, np.arange(len(ks)), 0)
    occ = np.arange(len(ks)) - np.maximum.accumulate(starts)
    # back to edge order
    occ_e = np.empty_like(occ)
    occ_e[o1] = occ

    ZR = PL - 1                                   # zero row (pad) per chunk
    gidx_all = []
    rank_e = pos[core_of_dst, chunk, dst_loc]     # rank of dst in (co,chunk)
    t_e = rank_e // 128
    p_e = rank_e % 128
    slot_e = (colbase[chunk, t_e] + occ_e) * 128 + p_e
    for co in range(cfg.cores):
        flat = np.full(S1, ZR, dtype=np.int16)
        m = core_of_dst == co
        flat[slot_e[m]] = src_loc[m].astype(np.int16)
        gidx_all.append(_wrap16(flat))

    # level-2: region r, natural tile t, k in {0,1} -> chunk 2r+k
    L2N = 2 * cfg.T * 2 * 128                     # slots per region
    l2_calls = []
    for r in range(2):
        t0 = 0
        while t0 < cfg.T:
            nt = min(cfg.TG, cfg.T - t0)
            l2_calls.append((r, t0, nt, r * L2N // 2 + t0 * 2 * 128))
            t0 += nt
    lidx_all = []
    nodes = np.arange(PL)
    for co in range(cfg.cores):
        flat = np.empty(L2N, dtype=np.int16)
        for r in range(2):
            for k in range(2):
                v = (k * PL + pos[co, 2 * r + k, nodes]).astype(np.int16)
                # slot ((t*2+k)*128 + p) ; node = t*128+p
                sl = ((nodes // 128) * 2 + k) * 128 + nodes % 128
                flat[r * (L2N // 2) + sl] = v
        lidx_all.append(_wrap16(flat))

    class Plan:
        pass
    plan = Plan()
    plan.calls = calls
    plan.l2 = l2_calls
    plan.S1 = S1
    plan.L2N = L2N
    plan.maxD = maxD
    plan.pad_frac = (S1 - int(np.sum(d)) / cfg.cores) / max(S1, 1)
    return plan, gidx_all, lidx_all


def host_prep(cfg, x, edge_index, W1, b1, W2, b2):
    N, H, F = cfg.N, cfg.H, cfg.F
    dst = np.asarray(edge_index[1], dtype=np.int64)
    deg = np.bincount(dst, minlength=N).astype(np.float64) + 1.0
    dinv = (1.0 / np.sqrt(deg)).astype(np.float32)

    plan, gidx_all, lidx_all = build_plan(cfg, edge_index)

    def tileize(v):  # [PL] -> [128, T]
        return np.ascontiguousarray(v.reshape(cfg.T, 128).T)

    xT = np.zeros((cfg.FP, cfg.PL), dtype=np.float32)
    W1p = np.zeros((cfg.FP, H), dtype=np.float32)
    W1p[:F] = W1.astype(np.float32)
    b1r = np.ascontiguousarray(np.broadcast_to(b1.astype(np.float32), (128, H)))
    b2r = np.ascontiguousarray(np.broadcast_to(b2.astype(np.float32), (128, cfg.O)))
    ident = np.eye(128, dtype=np.float32)

    in_maps = []
    for c in range(cfg.cores):
        xs = x[c * cfg.NP:(c + 1) * cfg.NP].astype(np.float32)
        xTc = xT.copy()
        xTc[:F, :cfg.NP] = xs.T
        dv = np.zeros(cfg.PL, dtype=np.float32)
        dv[:cfg.NP] = dinv[c * cfg.NP:(c + 1) * cfg.NP]
        rdv = np.zeros(cfg.PL, dtype=np.float32)
        rdv[:cfg.NP] = 1.0 / dv[:cfg.NP]
        in_maps.append({
            "xT": xTc,
            "w1": W1p,
            "b1r": b1r,
            "w2": W2.astype(np.float32),
            "b2r": b2r,
            "ident": ident,
            "dinv": tileize(dv),
            "dinvh": tileize((1.0 - cfg.alpha) * dv),
            "dinv2h": tileize((1.0 - cfg.alpha) * dv * dv),
            "rdinv": tileize(rdv),
            "gidx": gidx_all[c],
            "lidx": lidx_all[c],
        })
    return in_maps, plan


# ------------------------------------------------------------- graph builder

def build_graph(cfg, plan, compile_for_hw=True):
    import concourse.bass as bass
    import concourse.bacc as bacc
    import concourse.mybir as mybir
    from concourse.library_config import mlp

    f32 = mybir.dt.float32
    i16 = mybir.dt.int16
    H, O, T, PL = cfg.H, cfg.O, cfg.T, cfg.PL
    NS, NQ, LS, TG = cfg.NS, cfg.NQ, cfg.LS, cfg.TG
    S1, L2N = plan.S1, plan.L2N
    S16, L16 = S1 // 16, L2N // 16

    # round-robin main calls across chunks (queue = chunk)
    per_chunk = [[b for b in plan.calls if b[0] == c] for c in range(cfg.NCH)]
    order = []
    i = 0
    while any(i < len(pc) for pc in per_chunk):
        for c in range(cfg.NCH):
            if i < len(per_chunk[c]):
                order.append(per_chunk[c][i])
        i += 1
    CALLS = order
    NC = len(CALLS)
    # last issue position of a chunk-{0,1} call (region-A barrier point)
    last01 = max(k for k, b in enumerate(CALLS) if b[0] < 2)

    # level-2 calls interleaved A0,B0,A1,B1,... ; regions have equal length
    l2A = [b for b in plan.l2 if b[0] == 0]
    l2B = [b for b in plan.l2 if b[0] == 1]
    assert len(l2A) == len(l2B)
    L2C = [x for pair in zip(l2A, l2B) for x in pair]
    NL = len(L2C)
    NLR = len(l2A)         # calls per region

    # -------- static schedule: absolute per-slot semaphore counts ----------
    # main slots: gd (gather landed), dv (tree done), pw (partial written)
    gd_tot = [0] * NS
    dv_tot = [0] * NS
    pw_tot = [0] * NS
    lg_tot = [0] * (2 * LS)
    MAIN = []              # MAIN[t][k] = (b, s, gd_prior, dv_prior, pw_prior)
    BARA = []              # BARA[t] = (slot, pw_target) region-A barrier
    BARB = []              # BARB[t] = (slot, pw_target) region-B barrier
    L2S = []               # L2S[t][j] = (b, sl, lg_prior)
    for t in range(cfg.K):
        it = []
        qcnt = [0] * cfg.NCH
        for k, b in enumerate(CALLS):
            s = 2 * b[0] + qcnt[b[0]] % 2
            qcnt[b[0]] += 1
            it.append((b, s, gd_tot[s], dv_tot[s], pw_tot[s]))
            gd_tot[s] += 1
            if b[3] >= 2:
                dv_tot[s] += 1
            pw_tot[s] += 1
            if k == last01:
                BARA.append((s, pw_tot[s]))
        sB = it[-1][1]
        BARB.append((sB, pw_tot[sB]))
        MAIN.append(it)
        l2 = []
        for j, b in enumerate(L2C):
            r = b[0]
            jj = j // 2                       # index within region
            sl = r * LS + jj % LS
            l2.append((b, sl, lg_tot[sl]))
            lg_tot[sl] += 1
        L2S.append(l2)

    nc = bacc.Bacc("TRN2", target_bir_lowering=False, debug=False,
                   num_devices=cfg.cores, num_swdge_queues=NQ,
                   dynamic_dma_scratch_size=cfg.DMA_SCRATCH)

    xT_h = nc.declare_dram_parameter("xT", [cfg.FP, PL], f32, isOutput=False)
    w1_h = nc.declare_dram_parameter("w1", [cfg.FP, H], f32, isOutput=False)
    b1r_h = nc.declare_dram_parameter("b1r", [128, H], f32, isOutput=False)
    w2_h = nc.declare_dram_parameter("w2", [H, O], f32, isOutput=False)
    b2r_h = nc.declare_dram_parameter("b2r", [128, O], f32, isOutput=False)
    id_h = nc.declare_dram_parameter("ident", [128, 128], f32, isOutput=False)
    dinv_h = nc.declare_dram_parameter("dinv", [128, T], f32, isOutput=False)
    dinvh_h = nc.declare_dram_parameter("dinvh", [128, T], f32, isOutput=False)
    dinv2h_h = nc.declare_dram_parameter("dinv2h", [128, T], f32, isOutput=False)
    rdinv_h = nc.declare_dram_parameter("rdinv", [128, T], f32, isOutput=False)
    gidx_h = nc.declare_dram_parameter("gidx", [128, S16], i16, isOutput=False)
    lidx_h = nc.declare_dram_parameter("lidx", [128, L16], i16, isOutput=False)
    out_h = nc.declare_dram_parameter("out", [T, 128, O], f32, isOutput=True)

    cc_in = nc.dram_tensor("cc_in", [PL, H], f32)
    g_full = nc.dram_tensor("g_full", [cfg.GR, H], f32, addr_space="Shared")
    partials = nc.dram_tensor("partials", [cfg.NCH * PL, H], f32)

    # lin1 m-groups
    MGW = []
    rem = PL
    while rem > 0:
        w = min(512, rem)
        MGW.append(w)
        rem -= w

    rg = [list(range(cfg.cores))]

    ctxs = []

    def sb(name, shape, dtype):
        cm = nc.sbuf_tensor(name, shape, dtype)
        h = cm.__enter__()
        ctxs.append(cm)
        return h

    def ps(name, shape, dtype):
        cm = nc.psum_tensor(name, shape, dtype)
        h = cm.__enter__()
        ctxs.append(cm)
        return h

    def sem(name):
        cm = nc.semaphore(name)
        h = cm.__enter__()
        ctxs.append(cm)
        return h

    SLOT_F32 = cfg.SLOTC * H                      # per-partition f32 per slot
    stage = sb("stage", [128, NS * SLOT_F32], f32)
    # level-2 staging; doubles as lin1 xT staging during setup (2 x 2048 f32)
    l2st = sb("l2st", [128, 2 * LS * TG * 2 * H], f32)
    assert 2 * LS * TG * 2 * H >= 2 * 512 * cfg.KT
    idxg_sb = sb("idxg", [128, S16], i16)
    idxl_sb = sb("idxl", [128, L16], i16)
    g0a_sb = sb("g0a", [128, T, H], f32)
    g_sb = sb("g", [128, T, H], f32)
    msum_sb = sb("msum_sb", [128, T, H], f32)
    w1_sb = sb("w1_sb", [128, cfg.KT, H], f32)
    w2_sb = sb("w2_sb", [H, O], f32)
    b1r_sb = sb("b1r_sb", [128, H], f32)
    b2r_sb = sb("b2r_sb", [128, O], f32)
    id_sb = sb("id_sb", [128, 128], f32)
    dinv_sb = sb("dinv_sb", [128, T], f32)
    dinvh_sb = sb("dinvh_sb", [128, T], f32)
    dinv2h_sb = sb("dinv2h_sb", [128, T], f32)
    rdinv_sb = sb("rdinv_sb", [128, T], f32)
    tmp_sb = sb("tmp_sb", [128, 2, H], f32)
    u_sb = sb("u_sb", [128, 2, H], f32)
    ur_sb = sb("ur_sb", [128, 2, H], f32)
    ah_sb = sb("ah_sb", [128, 2, H], f32)
    lhsT_sb = sb("lhsT_sb", [H, 2, 128], f32)
    OG = 14                                   # out-group tiles
    out_sb = sb("out_sb", [128, 2, OG, O], f32)

    ps_mm = [ps("ps_mm0", [128, H], f32), ps("ps_mm1", [128, H], f32)]
    ps_tr = [ps("ps_tr0", [H, 128], f32), ps("ps_tr1", [H, 128], f32)]
    ps_o = [ps("ps_o0", [128, O], f32), ps("ps_o1", [128, O], f32)]

    s_in = sem("s_in")
    s_x = [sem("s_x0"), sem("s_x1")]
    s_mm = sem("s_mm")
    s_ppf = sem("s_ppf")
    s_ep = sem("s_ep")
    s_gw = sem("s_gw")
    s_cc = sem("s_cc")
    s_gd = [sem(f"s_gd{i}") for i in range(NS)]   # gather landed (16/call)
    s_dv = [sem(f"s_dv{i}") for i in range(NS)]   # DVE tree done (1/call)
    s_pw = [sem(f"s_pw{i}") for i in range(NS)]   # partial written (16/call)
    s_lg = [sem(f"s_lg{i}") for i in range(2 * LS)]  # L2 landed
    s_lc = [sem(f"s_lc{i}") for i in range(2 * LS)]  # L2 consumed
    s_gu = sem("s_gu")
    s_ah = sem("s_ah")
    s_u = sem("s_u")
    s_ur = sem("s_ur")
    s_tr = sem("s_tr")
    s_trc = sem("s_trc")
    s_mo = sem("s_mo")
    s_ob = sem("s_ob")
    s_ow = [sem("s_ow0"), sem("s_ow1")]

    def slot_view(s, nt, D):                  # [128, nt, D, H]
        base = s * SLOT_F32
        return stage[:, base:base + nt * D * H].rearrange(
            "p (n d m) -> p n d m", d=D, m=H)

    def slot_flat(s, ncols):                  # [128, ncols, H] gather target
        base = s * SLOT_F32
        return stage[:, base:base + ncols * H].rearrange(
            "p (n m) -> p n m", m=H)

    def l2_view(r, ls, nt):                   # [128, nt, 2, H]
        base = (r * LS + ls) * TG * 2 * H
        return l2st[:, base:base + nt * 2 * H].rearrange(
            "p (n k m) -> p n k m", k=2, m=H)

    def stage_slot_x(s, kt, w):               # [128, kt, w] lin1 xT view (l2st)
        return l2st[:, s * 512 * kt:(s + 1) * 512 * kt].rearrange(
            "p (k m) -> p k m", k=kt)[:, :, :w]

    xT3 = xT_h.ap().rearrange("(k p) m -> p k m", p=128)
    w13 = w1_h.ap().rearrange("(k p) m -> p k m", p=128)
    cc3 = cc_in.ap().rearrange("(n p) m -> p n m", p=128)

    n_in_dmas = 12

    tiles_per_group = [(w + 127) // 128 for w in MGW]
    cum_tiles = np.cumsum([0] + tiles_per_group)

    # ============================== setup block ==============================
    with nc.Block() as blk:
        @blk.sync
        def _(sy):
            sy.dma_start(idxg_sb[:, :], gidx_h[:, :]).then_inc(s_in, 16)
            sy.dma_start(idxl_sb[:, :], lidx_h[:, :]).then_inc(s_in, 16)
            sy.dma_start(w1_sb[:, :, :], w13).then_inc(s_in, 16)
            sy.dma_start(w2_sb[:, :], w2_h[:, :]).then_inc(s_in, 16)
            sy.dma_start(b1r_sb[:, :], b1r_h[:, :]).then_inc(s_in, 16)
            sy.dma_start(b2r_sb[:, :], b2r_h[:, :]).then_inc(s_in, 16)
            sy.dma_start(id_sb[:, :], id_h[:, :]).then_inc(s_in, 16)
            sy.dma_start(dinv_sb[:, :], dinv_h[:, :]).then_inc(s_in, 16)
            sy.dma_start(dinvh_sb[:, :], dinvh_h[:, :]).then_inc(s_in, 16)
            sy.dma_start(dinv2h_sb[:, :], dinv2h_h[:, :]).then_inc(s_in, 16)
            sy.dma_start(rdinv_sb[:, :], rdinv_h[:, :]).then_inc(s_in, 16)
            sy.wait_ge(s_in, 16 * (n_in_dmas - 1))
            col = 0
            for mg, w in enumerate(MGW):
                if mg >= 2:
                    sy.wait_ge(s_mm, int(cum_tiles[mg - 1]))
                sy.dma_start(stage_slot_x(mg % 2, cfg.KT, w),
                             xT3[:, :, col:col + w]).then_inc(s_x[mg % 2], 16)
                col += w
            sy.wait_ge(s_ep, T)
            sy.dma_start(cc3, g_sb[:, :, :]).then_inc(s_gw, 16)
            sy.wait_ge(s_gw, 16)

        @blk.tensor
        def _(pe):
            pe.wait_ge(s_in, 16 * (n_in_dmas - 1))
            ti = 0
            for mg, w in enumerate(MGW):
                pe.wait_ge(s_x[mg % 2], 16 * (mg // 2 + 1))
                nt = tiles_per_group[mg]
                for m in range(nt):
                    mw = min(128, w - m * 128)
                    if ti >= 2:
                        pe.wait_ge(s_ppf, ti - 1)
                    for k in range(cfg.KT):
                        ins = pe.matmul(
                            ps_mm[ti % 2][:mw, :],
                            stage_slot_x(mg % 2, cfg.KT, w)[:, k, m * 128:m * 128 + mw],
                            w1_sb[:, k, :],
                            start=(k == 0), stop=(k == cfg.KT - 1))
                        if k == cfg.KT - 1:
                            ins.then_inc(s_mm)
                    ti += 1

        @blk.vector
        def _(ve):
            ve.wait_ge(s_in, 16 * (n_in_dmas - 1))
            for ti in range(T):
                ve.wait_ge(s_mm, ti + 1)
                if ti >= 2:
                    ve.wait_ge(s_ep, ti - 1)   # Act consumed tmp slot
                ve.tensor_add(tmp_sb[:, ti % 2, :], ps_mm[ti % 2][:, :],
                              b1r_sb[:, :]).then_inc(s_ppf)

        @blk.scalar
        def _(ac):
            ac.wait_ge(s_in, 16 * (n_in_dmas - 1))
            for ti in range(T):
                ac.wait_ge(s_ppf, ti + 1)
                ac.activation(g0a_sb[:, ti, :], tmp_sb[:, ti % 2, :],
                              mybir.ActivationFunctionType.Copy,
                              scale=dinvh_sb[:, ti:ti + 1])
                ac.activation(g_sb[:, ti, :], tmp_sb[:, ti % 2, :],
                              mybir.ActivationFunctionType.Copy,
                              scale=dinv_sb[:, ti:ti + 1]).then_inc(s_ep)

        @blk.gpsimd
        def _(gp):
            gp.load_library(mlp)
            gp.wait_ge(s_gw, 16)
            gp.collective_compute(
                "AllGather", mybir.AluOpType.bypass, replica_groups=rg,
                ins=[cc_in.ap().opt()], outs=[g_full.ap().opt()],
            ).then_inc(s_cc)

    # ============================ iteration blocks ===========================
    for t in range(cfg.K):
        last = (t == cfg.K - 1)
        with nc.Block() as blk:
            @blk.gpsimd
            def _(gp, t=t, last=last):
                gp.wait_ge(s_cc, t + 1)
                # main gathers, queue = chunk
                for (c, coff, ncols, D, nt, t0), s, gdp, dvp, pwp in MAIN[t]:
                    if pwp > 0:
                        gp.wait_ge(s_pw[s], 16 * pwp)
                    nb = ncols * 128
                    off = coff * 128
                    gp.dma_gather(
                        slot_flat(s, ncols),
                        g_full[c * cfg.CH:(c + 1) * cfg.CH, :],
                        idxg_sb[:, off // 16:(off + nb) // 16],
                        nb, nb, H, elem_step=H, queue_num=c,
                        single_packet=False,
                    ).then_inc(s_gd[s], 16)
                # level-2 gathers: A on queues 0/1, B on 2/3, interleaved
                bar_done = [False, False]
                for j, ((r, t0, nt, loff), sl, lgp) in enumerate(L2S[t]):
                    if not bar_done[r]:
                        bs, btgt = (BARA[t] if r == 0 else BARB[t])
                        gp.wait_ge(s_pw[bs], 16 * btgt)
                        bar_done[r] = True
                    if lgp > 0:
                        gp.wait_ge(s_lc[sl], lgp)
                    nb = nt * 2 * 128
                    jj = j // 2
                    gp.dma_gather(
                        l2_view(r, sl - r * LS, nt).rearrange(
                            "p n k m -> p (n k) m"),
                        partials[2 * r * PL:(2 * r + 2) * PL, :],
                        idxl_sb[:, loff // 16:(loff + nb) // 16],
                        nb, nb, H, elem_step=H, queue_num=2 * r + (jj % 2),
                        single_packet=False,
                    ).then_inc(s_lg[sl], 16)
                if not last:
                    gp.wait_ge(s_gw, 16 * (t + 2))
                    gp.collective_compute(
                        "AllGather", mybir.AluOpType.bypass, replica_groups=rg,
                        ins=[cc_in.ap().opt()], outs=[g_full.ap().opt()],
                    ).then_inc(s_cc)

            @blk.vector
            def _(ve, t=t, last=last):
                # main tree reduces (issue order)
                for (c, coff, ncols, D, nt, t0), s, gdp, dvp, pwp in MAIN[t]:
                    ve.wait_ge(s_gd[s], 16 * (gdp + 1))
                    if D >= 2:
                        v = slot_view(s, nt, D)
                        cur = D
                        ins = None
                        while cur > 1:
                            h = cur // 2
                            lo = cur - h
                            ins = ve.tensor_add(v[:, :, 0:h, :],
                                                v[:, :, 0:h, :],
                                                v[:, :, lo:lo + h, :])
                            cur = lo
                        ins.then_inc(s_dv[s])
                # level-2 combines + self loop + g update
                for j, ((r, t0, nt, loff), sl, lgp) in enumerate(L2S[t]):
                    ve.wait_ge(s_lg[sl], 16 * (lgp + 1))
                    v = l2_view(r, sl - r * LS, nt)
                    mr = msum_sb[:, t0:t0 + nt, :]
                    if r == 0:
                        ve.tensor_add(mr, v[:, :, 0, :],
                                      v[:, :, 1, :]).then_inc(s_lc[sl])
                    else:
                        ve.tensor_add(mr, mr, v[:, :, 0, :])
                        ve.tensor_add(mr, mr, v[:, :, 1, :]).then_inc(s_lc[sl])
                        ve.tensor_add(mr, mr, g_sb[:, t0:t0 + nt, :])
                        if not last:
                            for ti in range(t0, t0 + nt):
                                ins = ve.scalar_tensor_tensor(
                                    g_sb[:, ti, :], msum_sb[:, ti, :],
                                    dinv2h_sb[:, ti:ti + 1], g0a_sb[:, ti, :],
                                    mybir.AluOpType.mult, mybir.AluOpType.add)
                                if ti == T - 1:
                                    ins.then_inc(s_gu)
                        else:
                            if t0 + nt == T:
                                ve.tensor_copy(tmp_sb[:, 0, :],
                                               msum_sb[:, 0, :]).then_inc(s_gu)

            @blk.sync
            def _(sy, t=t, last=last):
                # partial write-outs (issue order; HWDGE FIFO per engine)
                for (c, coff, ncols, D, nt, t0), s, gdp, dvp, pwp in MAIN[t]:
                    if D >= 2:
                        sy.wait_ge(s_dv[s], dvp + 1)
                    else:
                        sy.wait_ge(s_gd[s], 16 * (gdp + 1))
                    v = slot_view(s, nt, D)
                    dst = partials[c * PL + t0 * 128:
                                   c * PL + (t0 + nt) * 128, :].rearrange(
                                       "(n p) m -> p n m", p=128)
                    sy.dma_start(dst, v[:, :, 0, :]).then_inc(s_pw[s], 16)
                sy.wait_ge(s_gu, t + 1)
                if not last:
                    sy.dma_start(cc3, g_sb[:, :, :]).then_inc(s_gw, 16)
                    sy.wait_ge(s_gw, 16 * (t + 2))

    # ============================== epilogue =================================
    with nc.Block() as blk:
        @blk.scalar
        def _(ac):
            for ti in range(T):
                if ti >= 2:
                    ac.wait_ge(s_u, ti - 1)     # DVE consumed ah slot
                ac.activation(ah_sb[:, ti % 2, :], g0a_sb[:, ti, :],
                              mybir.ActivationFunctionType.Copy,
                              scale=rdinv_sb[:, ti:ti + 1]).then_inc(s_ah)
                ac.wait_ge(s_u, ti + 1)
                if ti >= 2:
                    ac.wait_ge(s_tr, ti - 1)    # PE consumed ur slot
                ac.activation(ur_sb[:, ti % 2, :], u_sb[:, ti % 2, :],
                              mybir.ActivationFunctionType.Relu).then_inc(s_ur)

        @blk.vector
        def _(ve):
            ve.wait_ge(s_gu, cfg.K)
            for ti in range(T):
                og, oslot = ti // OG, (ti // OG) % 2
                if ti % OG == 0 and og >= 2:
                    ve.wait_ge(s_ow[og % 2], 16 * (og // 2))
                ve.wait_ge(s_ah, ti + 1)
                if ti >= 2:
                    ve.wait_ge(s_ur, ti - 1)    # Act consumed u slot
                ve.scalar_tensor_tensor(
                    u_sb[:, ti % 2, :], msum_sb[:, ti, :],
                    dinvh_sb[:, ti:ti + 1], ah_sb[:, ti % 2, :],
                    mybir.AluOpType.mult, mybir.AluOpType.add).then_inc(s_u)
                ve.wait_ge(s_tr, ti + 1)
                ve.tensor_copy(lhsT_sb[:, ti % 2, :],
                               ps_tr[ti % 2][:, :]).then_inc(s_trc)
                ve.wait_ge(s_mo, ti + 1)
                ve.tensor_add(out_sb[:, oslot, ti % OG, :], ps_o[ti % 2][:, :],
                              b2r_sb[:, :]).then_inc(s_ob)

        @blk.tensor
        def _(pe):
            for ti in range(T):
                pe.wait_ge(s_ur, ti + 1)
                if ti >= 2:
                    pe.wait_ge(s_trc, ti - 1)
                pe.transpose(ps_tr[ti % 2][:, :], ur_sb[:, ti % 2, :],
                             id_sb[:, :]).then_inc(s_tr)
                pe.wait_ge(s_trc, ti + 1)
                if ti >= 2:
                    pe.wait_ge(s_ob, ti - 1)
                pe.matmul(ps_o[ti % 2][:, :], lhsT_sb[:, ti % 2, :],
                          w2_sb[:, :], start=True, stop=True).then_inc(s_mo)

        @blk.sync
        def _(sy):
            ngroups = (T + OG - 1) // OG
            for og in range(ngroups):
                t0 = og * OG
                nt = min(OG, T - t0)
                sy.wait_ge(s_ob, t0 + nt)
                dst = out_h[t0:t0 + nt, :, :].rearrange("n p m -> p n m")
                sy.dma_start(dst, out_sb[:, og % 2, 0:nt, :]).then_inc(
                    s_ow[og % 2], 16)
            for par in range(2):
                n_par = (ngroups + 1 - par) // 2
                if n_par:
                    sy.wait_ge(s_ow[par], 16 * n_par)

    print(f"SBUF used: {(nc.sbuf_base + (nc.SBUF_PARTITION_SIZE_BYTES - nc.sbuf_top)) / 1024:.0f} KB/part "
          f"(base {nc.sbuf_base//1024}KB top-res {(nc.SBUF_PARTITION_SIZE_BYTES - nc.sbuf_top)//1024}KB of {nc.SBUF_PARTITION_SIZE_BYTES//1024}KB) "
          f"NC={NC} NL={NL} S1={S1} pad={plan.pad_frac:.3f} maxD={plan.maxD}")
    if compile_for_hw:
        nc.compile()
    return nc


# ----------------------------------------------------------------- kernel()

_CACHE = {}


def _run(cfg, inputs, trace=False):
    from concourse.bass_utils import run_bass_kernel_spmd

    in_maps, plan = host_prep(cfg, inputs["x"], inputs["edge_index"],
                              inputs["W1"], inputs["b1"],
                              inputs["W2"], inputs["b2"])
    key = (cfg.N, cfg.E, plan.S1, tuple(b[:4] for b in plan.calls))
    if key not in _CACHE:
        _CACHE[key] = build_graph(cfg, plan)
    nc = _CACHE[key]
    res = run_bass_kernel_spmd(nc, in_maps, list(range(cfg.cores)), trace=trace)
    outs = []
    for c in range(cfg.cores):
        o = np.asarray(res.results[c]["out"]).reshape(cfg.PL, cfg.O)
        outs.append(o[:cfg.NP])
    return np.concatenate(outs, axis=0), res


def kernel(**inputs):
    out, _ = _run(FULL, inputs)
    return out
